# revision 1
# baseline (speedup 1.0000x reference)
"""Trainium2 Bass kernel for nn_AttentionBlock (GroupNorm -> 1x1 qkv conv ->
softmax attention over N=HW -> 1x1 proj -> residual).

Sharding: 8 cores = 4 images x 2 query-column halves. Each core receives its
image column-permuted so its own 2048 query columns come first; attention is
permutation-invariant over key/value positions, so k/v use all 4096 columns
in permuted order. GroupNorm stats are computed on-chip per core (full image).

Math folding done on host (tiny O(C^2) numpy):
  - gn_w folded into qkv weight columns; gn_b folded into qkv biases.
  - 1/sqrt(C) score scale folded into Wq and bq.
  - k bias dropped entirely (adds a per-row constant to scores: softmax-invariant).
  - v bias folded into proj bias (softmax rows sum to 1): bp_eff = bp + Wp @ bv.
On-chip per core:
  h = (x - mean_g) * rstd_g              (per-channel affine from group stats)
  q = Wq^T-matmul(h) + bq  (cols 0:2048) ; k = Wk-matmul(h) (all cols)
  vpos[m, c] = Wv-matmul(h)              (position-major layout)
  per 512-col tile of q:  E[m, n] = exp(k_chunk^T q_tile) accumulated flash-style:
     av[c, n] += vpos_chunk^T E ;  eacc[n] += E (DVE)
  S = ones^T eacc (all-ones 128x128 matmul -> S replicated on all partitions)
  ha = av * (1/S) ; y = x_tile + Wp-matmul(ha) + bp_eff
"""

import numpy as np

B, C, HH, WW = 4, 256, 64, 64
N = HH * WW            # 4096
NH = N // 2            # 2048 query columns per core
GROUPS = 32
GSIZE = C // GROUPS    # 8
EPS = 1e-5
NCORES = 8
P = 128
NT = NH // 512         # 4 query tiles per core
MC = N // P            # 32 key chunks
KT = N // 512          # 8 column tiles for k

_prog = None


def _build_program():
    import concourse.bacc as bacc
    import concourse.tile as tile
    from concourse import mybir

    f32 = mybir.dt.float32
    f32r = mybir.dt.float32r
    AF = mybir.ActivationFunctionType
    ALU = mybir.AluOpType

    nc = bacc.Bacc("TRN2", target_bir_lowering=False, debug=False,
                   num_devices=NCORES)

    x_d = nc.dram_tensor("x", [C, N], f32, kind="ExternalInput").ap()
    wqk_d = nc.dram_tensor("wqk", [C, 2 * C], f32r, kind="ExternalInput").ap()
    wv_d = nc.dram_tensor("wv", [C, C], f32r, kind="ExternalInput").ap()
    wp_d = nc.dram_tensor("wp", [C, C], f32r, kind="ExternalInput").ap()
    bq_d = nc.dram_tensor("bq", [C, 1], f32, kind="ExternalInput").ap()
    bp_d = nc.dram_tensor("bp", [C, 1], f32, kind="ExternalInput").ap()
    gm_d = nc.dram_tensor("gm", [P, 16], f32, kind="ExternalInput").ap()
    gt_d = nc.dram_tensor("gt", [16, P], f32, kind="ExternalInput").ap()
    onr_d = nc.dram_tensor("onr", [P, P], f32r, kind="ExternalInput").ap()
    y_d = nc.dram_tensor("y", [C, NH], f32, kind="ExternalOutput").ap()

    xv = x_d.rearrange("(j p) n -> p j n", p=P)        # [128, 2, 4096]
    wqkv = wqk_d.rearrange("(j p) o -> p j o", p=P)    # [128, 2, 512]
    wvv = wv_d.rearrange("(j p) o -> p j o", p=P)
    wpv = wp_d.rearrange("(j p) o -> p j o", p=P)
    bqv = bq_d.rearrange("(j p) o -> p j o", p=P)      # [128, 2, 1]
    bpv = bp_d.rearrange("(j p) o -> p j o", p=P)
    yv = y_d.rearrange("(j p) n -> p j n", p=P)        # [128, 2, 2048]

    with tile.TileContext(nc) as tc:
        with (
            tc.tile_pool(name="big", bufs=1) as big,
            tc.tile_pool(name="wts", bufs=1) as wts,
            tc.tile_pool(name="stats", bufs=1) as stats,
            tc.tile_pool(name="epool", bufs=6) as epool,
            tc.tile_pool(name="acc", bufs=2) as accp,
            tc.tile_pool(name="rp", bufs=2) as rp,
            tc.tile_pool(name="hap", bufs=2) as hap,
            tc.tile_pool(name="yp", bufs=2) as yp,
        ):

            # PE warmup: dense dummy matmuls fill the x-DMA wait so the HAM
            # clock gate opens (K=8/8) before the real matmul stream starts.
            dummy = wts.tile([P, 512], f32)
            nc.vector.memset(dummy, 0.0)
            with tc.tile_pool(name="psW", bufs=1, space="PSUM") as psw:
                wps = psw.tile([P, 512], f32, tag="w")
                dr = dummy.bitcast(f32r)
                for _ in range(82):
                    nc.tensor.matmul(wps, lhsT=dr[:, 0:P], rhs=dr,
                                     start=True, stop=True)

            # ---- load x first (critical path), 3 parallel DMA queues ----
            xs = big.tile([P, 2, N], f32)
            dma_engs = [nc.sync, nc.gpsimd, nc.scalar, nc.sync]
            for j in range(2):
                for qd in range(4):
                    sl = slice(qd * 1024, (qd + 1) * 1024)
                    dma_engs[(j * 4 + qd) % 3].dma_start(
                        out=xs[:, j, sl], in_=xv[:, j, sl])

            # ---- weights / consts (off the critical path) ----
            wqk = wts.tile([P, 2, 2 * C], f32r)
            nc.gpsimd.dma_start(out=wqk, in_=wqkv)
            wv = wts.tile([P, 2, C], f32r)
            nc.scalar.dma_start(out=wv, in_=wvv)
            wp = wts.tile([P, 2, C], f32r)
            nc.scalar.dma_start(out=wp, in_=wpv)
            bq = wts.tile([P, 2, 1], f32)
            nc.sync.dma_start(out=bq, in_=bqv)
            bp = wts.tile([P, 2, 1], f32)
            nc.sync.dma_start(out=bp, in_=bpv)
            gm = wts.tile([P, 16], f32)
            nc.sync.dma_start(out=gm, in_=gm_d)
            gt = wts.tile([16, P], f32)
            nc.sync.dma_start(out=gt, in_=gt_d)
            ones_sq = wts.tile([P, P], f32r)
            nc.sync.dma_start(out=ones_sq, in_=onr_d)
            eps_t = wts.tile([16, 1], f32)
            nc.vector.memset(eps_t, EPS)

            # ---- group stats ----
            AB = stats.tile([P, 2, 2], f32)  # per-channel (mean, rstd)
            with tc.tile_pool(name="psStat", bufs=1, space="PSUM") as psst:
                for j in range(2):
                    st6 = stats.tile([P, 8, 6], f32, tag="st6")
                    xsr = xs[:, j, :].rearrange("p (s f) -> p s f", f=512)
                    for sg in range(8):
                        nc.vector.bn_stats(out=st6[:, sg, :], in_=xsr[:, sg, :])
                    mv = stats.tile([P, 2], f32, tag="mv")
                    nc.vector.bn_aggr(out=mv, in_=st6)
                    # t2 = (mean, var + mean^2)
                    t2 = stats.tile([P, 2], f32, tag="t2")
                    nc.vector.tensor_copy(out=t2[:, 0:1], in_=mv[:, 0:1])
                    nc.vector.scalar_tensor_tensor(
                        out=t2[:, 1:2], in0=mv[:, 0:1], scalar=mv[:, 0:1],
                        in1=mv[:, 1:2], op0=ALU.mult, op1=ALU.add,
                    )
                    gagg = psst.tile([16, 2], f32, tag="gagg")
                    nc.tensor.matmul(gagg, lhsT=gm, rhs=t2, start=True, stop=True)
                    # grs = (gmean, rstd)
                    grs = stats.tile([16, 2], f32, tag="grs")
                    nc.scalar.copy(out=grs[:, 0:1], in_=gagg[:, 0:1])
                    sq = stats.tile([16, 1], f32, tag="sq")
                    nc.scalar.square(out=sq, in_=gagg[:, 0:1])
                    var = stats.tile([16, 1], f32, tag="var")
                    nc.vector.tensor_sub(out=var, in0=gagg[:, 1:2], in1=sq)
                    nc.scalar.activation(out=var, in_=var, func=AF.Sqrt,
                                         bias=eps_t, scale=1.0)
                    nc.vector.reciprocal(out=grs[:, 1:2], in_=var)
                    gb = psst.tile([P, 2], f32, tag="gb")
                    nc.tensor.matmul(gb, lhsT=gt, rhs=grs, start=True, stop=True)
                    nc.scalar.copy(out=AB[:, j, :], in_=gb)

            # bridge the PE clock gate through the normalize (DVE) phase
            with tc.tile_pool(name="psW2", bufs=1, space="PSUM") as psw2:
                wps2 = psw2.tile([P, 512], f32, tag="w2")
                dr2 = dummy.bitcast(f32r)
                for _ in range(25):
                    nc.tensor.matmul(wps2, lhsT=dr2[:, 0:P], rhs=dr2,
                                     start=True, stop=True)

            # ---- qkv ----
            q_s = big.tile([P, 2, NH], f32r)
            k_s = big.tile([P, 2, N], f32r)
            v_s = big.tile([P, MC, C], f32r)
            with (
                tc.tile_pool(name="hp", bufs=1) as hp,
                tc.tile_pool(name="psD", bufs=4, space="PSUM") as psd,
            ):
                hs = hp.tile([P, 2, N], f32r)
                for j in range(2):
                    for nd in range(4):
                        ns = slice(nd * 1024, (nd + 1) * 1024)
                        nc.vector.tensor_scalar(
                            out=hs[:, j, ns], in0=xs[:, j, ns],
                            scalar1=AB[:, j, 0:1], scalar2=AB[:, j, 1:2],
                            op0=ALU.subtract, op1=ALU.mult,
                        )
                # q (own half) and k (all columns)
                for jo in range(2):
                    for tt in range(NT):
                        sl = slice(tt * 512, (tt + 1) * 512)
                        ps = psd.tile([P, 512], f32, tag="mm")
                        for j in range(2):
                            nc.tensor.matmul(
                                ps, lhsT=wqk[:, j, jo * P:(jo + 1) * P],
                                rhs=hs[:, j, sl],
                                start=(j == 0), stop=(j == 1),
                            )
                        nc.vector.tensor_scalar_add(out=q_s[:, jo, sl],
                                                    in0=ps,
                                                    scalar1=bq[:, jo, :])
                for jo in range(2):
                    for tt in range(KT):
                        sl = slice(tt * 512, (tt + 1) * 512)
                        ps = psd.tile([P, 512], f32, tag="mm")
                        for j in range(2):
                            nc.tensor.matmul(
                                ps, lhsT=wqk[:, j, C + jo * P:C + (jo + 1) * P],
                                rhs=hs[:, j, sl],
                                start=(j == 0), stop=(j == 1),
                            )
                        if tt % 2 == 0:
                            nc.scalar.copy(out=k_s[:, jo, sl], in_=ps)
                        else:
                            nc.vector.tensor_copy(out=k_s[:, jo, sl], in_=ps)
                # vpos[m, c]
                for mc in range(MC):
                    msl = slice(mc * P, (mc + 1) * P)
                    ps = psd.tile([P, 512], f32, tag="mm")
                    for j in range(2):
                        nc.tensor.matmul(
                            ps[:, 0:C], lhsT=hs[:, j, msl], rhs=wv[:, j, :],
                            start=(j == 0), stop=(j == 1),
                        )
                    if mc % 2 == 0:
                        nc.scalar.copy(out=v_s[:, mc, :], in_=ps[:, 0:C])
                    else:
                        nc.vector.tensor_copy(out=v_s[:, mc, :], in_=ps[:, 0:C])

            # ---- attention ----
            with (
                tc.tile_pool(name="psQK", bufs=3, space="PSUM") as psqk,
                tc.tile_pool(name="psAV", bufs=2, space="PSUM") as psav,
                tc.tile_pool(name="psSP", bufs=1, space="PSUM") as pssp,
            ):
                # Tile tails (S -> recip -> ha -> proj -> y) are emitted
                # INSIDE the next tile's mc loop: the PE executes in emission
                # order, so interleaving lets next-tile qk/av matmuls cover
                # the DVE recip/ha latency instead of stalling at boundaries.
                def tail_stage1(av0, av1, ea, st):
                    # S matmuls + recip + ha scale (PE 2 MMs + DVE work)
                    sps = pssp.tile([P, 512], f32, name="sps", tag="sp")
                    nc.tensor.matmul(sps, lhsT=ones_sq, rhs=ea[0],
                                     start=True, stop=False)
                    nc.tensor.matmul(sps, lhsT=ones_sq, rhs=ea[1],
                                     start=False, stop=True)
                    rb = rp.tile([P, 512], f32, name="rb", tag="rb")
                    nc.vector.reciprocal(out=rb, in_=sps)
                    ha = hap.tile([P, 2, 512], f32r, name="ha", tag="ha")
                    nc.vector.tensor_mul(out=ha[:, 0, :], in0=av0, in1=rb)
                    nc.vector.tensor_mul(out=ha[:, 1, :], in0=av1, in1=rb)
                    st["ha"] = ha

                def tail_stage2(st, psl):
                    ha = st["ha"]
                    yt = yp.tile([P, 2, 512], f32, name="yt", tag="yt")
                    for jo in range(2):
                        pp = pssp.tile([P, 512], f32, name="pp", tag="sp")
                        for j in range(2):
                            nc.tensor.matmul(
                                pp, lhsT=wp[:, j, jo * P:(jo + 1) * P],
                                rhs=ha[:, j, :],
                                start=(j == 0), stop=(j == 1),
                            )
                        nc.vector.scalar_tensor_tensor(
                            out=yt[:, jo, :], in0=pp, scalar=bp[:, jo, :],
                            in1=xs[:, jo, psl], op0=ALU.add, op1=ALU.add,
                        )
                    nc.sync.dma_start(out=yv[:, :, psl], in_=yt)

                pend = None
                for tt in range(NT):
                    sl = slice(tt * 512, (tt + 1) * 512)
                    # two interleaved exp-sum accumulators (halves the RAW chain)
                    ea = [accp.tile([P, 512], f32r, name=f"eacc{i}", tag=f"eacc{i}")
                          for i in range(2)]
                    nc.vector.memset(ea[0].bitcast(f32), 0.0)
                    nc.vector.memset(ea[1].bitcast(f32), 0.0)
                    av0 = psav.tile([P, 512], f32, name="av0", tag="av0")
                    av1 = psav.tile([P, 512], f32, name="av1", tag="av1")
                    # one-stage software pipeline: av[mc-1] runs while
                    # exp[mc] computes, so the PE never waits on the ACT.
                    ets = [None] * MC

                    def av_pair(mc, av0=av0, av1=av1, ea=ea, ets=ets):
                        et = ets[mc]
                        nc.tensor.matmul(av0, lhsT=v_s[:, mc, 0:P], rhs=et,
                                         start=(mc == 0), stop=(mc == MC - 1))
                        nc.tensor.matmul(av1, lhsT=v_s[:, mc, P:C], rhs=et,
                                         start=(mc == 0), stop=(mc == MC - 1))
                        acc = ea[mc % 2]
                        nc.vector.tensor_add(out=acc, in0=acc.bitcast(f32),
                                             in1=et.bitcast(f32))

                    for mc in range(MC):
                        msl = slice(mc * P, (mc + 1) * P)
                        qk = psqk.tile([P, 512], f32, name="qk", tag="qk")
                        for j in range(2):
                            nc.tensor.matmul(
                                qk, lhsT=k_s[:, j, msl], rhs=q_s[:, j, sl],
                                start=(j == 0), stop=(j == 1),
                            )
                        et = epool.tile([P, 512], f32r, name=f"et{mc % 6}",
                                        tag="et")
                        ets[mc] = et
                        nc.scalar.activation(out=et, in_=qk, func=AF.Exp)
                        if mc > 0:
                            av_pair(mc - 1)
                        if pend is not None:
                            if mc == 2:
                                tail_stage1(*pend[:3], pend[3])
                            elif mc == 8:
                                tail_stage2(pend[3], pend[4])
                                pend = None
                    av_pair(MC - 1)
                    pend = (av0, av1, ea, {}, sl)
                # last tile: split the tail per 256-col half so the DVE
                # recip/scale of half 1 overlaps the PE proj of half 0
                lav0, lav1, lea, _, lsl = pend
                sps = pssp.tile([P, 512], f32, name="sps_l", tag="sp")
                nc.tensor.matmul(sps, lhsT=ones_sq, rhs=lea[0],
                                 start=True, stop=False)
                nc.tensor.matmul(sps, lhsT=ones_sq, rhs=lea[1],
                                 start=False, stop=True)
                yt = yp.tile([P, 2, 512], f32, name="yt_l", tag="yt")
                for h in range(2):
                    hsl = slice(h * 256, (h + 1) * 256)
                    osl = slice(lsl.start + h * 256, lsl.start + (h + 1) * 256)
                    rbh = rp.tile([P, 256], f32, name=f"rbh{h}", tag="rb")
                    nc.vector.reciprocal(out=rbh, in_=sps[:, hsl])
                    hah = hap.tile([P, 2, 256], f32r, name=f"hah{h}", tag="ha")
                    nc.vector.tensor_mul(out=hah[:, 0, :], in0=lav0[:, hsl],
                                         in1=rbh)
                    nc.vector.tensor_mul(out=hah[:, 1, :], in0=lav1[:, hsl],
                                         in1=rbh)
                    for jo in range(2):
                        pp = psqk.tile([P, 256], f32, name="pp_l", tag="qk")
                        for j in range(2):
                            nc.tensor.matmul(
                                pp, lhsT=wp[:, j, jo * P:(jo + 1) * P],
                                rhs=hah[:, j, :],
                                start=(j == 0), stop=(j == 1),
                            )
                        nc.vector.scalar_tensor_tensor(
                            out=yt[:, jo, hsl], in0=pp, scalar=bp[:, jo, :],
                            in1=xs[:, jo, osl], op0=ALU.add, op1=ALU.add,
                        )
                nc.sync.dma_start(out=yv[:, :, lsl], in_=yt)

    nc.compile()
    return nc


def _get_prog():
    global _prog
    if _prog is None:
        _prog = _build_program()
    return _prog


def _host_prep(x, gn_w, gn_b, qkv_w, qkv_b, proj_w, proj_b):
    """Returns (shared input dict, per-core x list)."""
    x = np.asarray(x, dtype=np.float32)
    gn_w = np.asarray(gn_w, dtype=np.float32)
    gn_b = np.asarray(gn_b, dtype=np.float32)
    qkv_w = np.asarray(qkv_w, dtype=np.float32)
    qkv_b = np.asarray(qkv_b, dtype=np.float32)
    proj_w = np.asarray(proj_w, dtype=np.float32)
    proj_b = np.asarray(proj_b, dtype=np.float32)

    scale = 1.0 / np.sqrt(C).astype(np.float32)
    Wq = qkv_w[0:C] * gn_w[None, :] * scale
    bq_eff = (qkv_w[0:C] @ gn_b + qkv_b[0:C]) * scale
    Wk = qkv_w[C:2 * C] * gn_w[None, :]
    Wv = qkv_w[2 * C:3 * C] * gn_w[None, :]
    bv_eff = qkv_w[2 * C:3 * C] @ gn_b + qkv_b[2 * C:3 * C]
    bp_eff = proj_b + proj_w @ bv_eff

    wqk = np.concatenate([Wq.T, Wk.T], axis=1).astype(np.float32)  # [C, 2C]
    wv_h = np.ascontiguousarray(Wv.T, dtype=np.float32)
    wp_h = np.ascontiguousarray(proj_w.T, dtype=np.float32)

    cidx = np.arange(P)
    gm = np.zeros((P, 16), dtype=np.float32)
    gm[cidx, cidx // GSIZE] = 1.0 / GSIZE
    gt = np.zeros((16, P), dtype=np.float32)
    gt[cidx // GSIZE, cidx] = 1.0

    shared = {
        "onr": np.ones((P, P), dtype=np.float32),
        "wqk": wqk,
        "wv": wv_h,
        "wp": wp_h,
        "bq": bq_eff.reshape(C, 1).astype(np.float32),
        "bp": bp_eff.reshape(C, 1).astype(np.float32),
        "gm": gm,
        "gt": gt,
    }

    xf = x.reshape(B, C, N)
    xs_per_core = []
    for core in range(NCORES):
        b, half = core // 2, core % 2
        if half == 0:
            xc = xf[b]
        else:
            xc = np.concatenate([xf[b][:, NH:], xf[b][:, :NH]], axis=1)
        xs_per_core.append(np.ascontiguousarray(xc))
    return shared, xs_per_core


def run_sharded(inputs, trace=False, trace_kwargs=None):
    """Run the 8-core kernel. Returns (full_output, BassKernelResults)."""
    from concourse.bass_utils import run_bass_kernel_spmd

    nc = _get_prog()
    shared, xs_per_core = _host_prep(**inputs)
    in_maps = [{**shared, "x": xs_per_core[c]} for c in range(NCORES)]
    kw = {}
    if trace:
        kw["trace"] = True
        if trace_kwargs:
            kw["trace_kwargs"] = trace_kwargs
    res = run_bass_kernel_spmd(nc, in_maps, list(range(NCORES)), **kw)

    out = np.empty((B, C, N), dtype=np.float32)
    for core in range(NCORES):
        b, half = core // 2, core % 2
        yc = res.results[core]["y"]
        out[b][:, half * NH:(half + 1) * NH] = yc
    return out.reshape(B, C, HH, WW), res


def kernel(**inputs):
    out, _ = run_sharded(inputs)
    return out



# revision 2
# speedup vs baseline: 1.2991x; 1.2991x over previous
"""Trainium2 Bass kernel for nn_AttentionBlock (GroupNorm -> 1x1 qkv conv ->
softmax attention over N=HW -> 1x1 proj -> residual).

Sharding: 8 cores = 4 images x 2 query-column halves. Each core receives its
image column-permuted so its own 2048 query columns come first; attention is
permutation-invariant over key/value positions, so k/v use all 4096 columns
in permuted order.

v2 design (fp8 DoubleRow attention):
  - x uploaded twice: bf16 (stats + residual) and fp8e4m3 (matmul operand).
  - GroupNorm is folded into the qkv weights at runtime: wqk_s = wqk * r_c,
    so no normalize pass over x is needed (q/k additionally carry a x4 scale
    for fp8 health, compensated in the exp input scale 1/(16*4*4) = 1/256).
    The -mu*r correction is a per-output-channel constant: for k it is
    softmax-invariant (dropped), for q it folds into the q bias (on-chip
    matvec), for v it folds into the proj bias (on-chip matvec chain).
  - q/k/v stored fp8 in [128, 2, n] channel-pair layout; all attention
    matmuls (qk, av, S) run fp8 DoubleRow (contraction 256 in one pass,
    0.5 cycles/row).
  - The softmax denominator S is accumulated on the PE with an all-ones fp8
    DoubleRow matmul into a PSUM bank (replicated across partitions), not on
    DVE (the old eacc DVE adds were ~80us of DVE time).
  - exp runs on ACT per PAIR of key chunks ([128, 2, 512] PSUM read) to
    amortize per-instruction overhead; ACT does nothing else in steady state.
  - proj stays bf16 (ha ~ softmax average is too small for fp8).
  - y stored bf16, upcast on host.
"""

import numpy as np

B, C, HH, WW = 4, 256, 64, 64
N = HH * WW            # 4096
NH = N // 2            # 2048 query columns per core
GROUPS = 32
GSIZE = C // GROUPS    # 8
EPS = 1e-5
NCORES = 8
P = 128
NT = NH // 512         # 4 query tiles per core
MC = N // P            # 32 key chunks
MCP = MC // 2          # 16 key-chunk pairs
KT = N // 512          # 8 column tiles for k
SQ = 4.0               # q,k fp8 pre-scale (folded into wqk host upload)
MUS = 32.0             # mu fp8 pre-scale (compensated in bias fixups)

_prog = None


def _build_program():
    import concourse.bacc as bacc
    import concourse.tile as tile
    from concourse import mybir

    f32 = mybir.dt.float32
    bf16 = mybir.dt.bfloat16
    f8 = mybir.dt.float8e4
    AF = mybir.ActivationFunctionType
    ALU = mybir.AluOpType
    DR = mybir.MatmulPerfMode.DoubleRow

    nc = bacc.Bacc("TRN2", target_bir_lowering=False, debug=False,
                   num_devices=NCORES)

    xb_d = nc.dram_tensor("xb", [C, N], bf16, kind="ExternalInput").ap()
    x8_d = nc.dram_tensor("x8", [C, N], f8, kind="ExternalInput").ap()
    wqk_d = nc.dram_tensor("wqk", [C, 2 * C], bf16, kind="ExternalInput").ap()
    wv_d = nc.dram_tensor("wv", [C, C], bf16, kind="ExternalInput").ap()
    wp_d = nc.dram_tensor("wp", [C, C], bf16, kind="ExternalInput").ap()
    bq4_d = nc.dram_tensor("bq4", [C, 1], f32, kind="ExternalInput").ap()
    bp0_d = nc.dram_tensor("bp0", [C, 1], f32, kind="ExternalInput").ap()
    gm_d = nc.dram_tensor("gm", [P, 16], f32, kind="ExternalInput").ap()
    gt_d = nc.dram_tensor("gt", [16, P], f32, kind="ExternalInput").ap()
    y_d = nc.dram_tensor("y", [C, NH], bf16, kind="ExternalOutput").ap()

    xbv = xb_d.rearrange("(j p) n -> p j n", p=P)      # [128, 2, 4096]
    x8v = x8_d.rearrange("(j p) n -> p j n", p=P)
    wqkv = wqk_d.rearrange("(j p) o -> p j o", p=P)    # [128, 2, 512]
    wvv = wv_d.rearrange("(j p) o -> p j o", p=P)
    wpv = wp_d.rearrange("(j p) o -> p j o", p=P)
    bq4v = bq4_d.rearrange("(j p) o -> p j o", p=P)    # [128, 2, 1]
    bp0v = bp0_d.rearrange("(j p) o -> p j o", p=P)
    yv = y_d.rearrange("(j p) n -> p j n", p=P)        # [128, 2, 2048]

    with tile.TileContext(nc) as tc:
        with (
            tc.tile_pool(name="big", bufs=1) as big,
            tc.tile_pool(name="wts", bufs=1) as wts,
            tc.tile_pool(name="stats", bufs=1) as stats,
            tc.tile_pool(name="epool", bufs=3) as epool,
            tc.tile_pool(name="rp", bufs=2) as rp,
            tc.tile_pool(name="hap", bufs=2) as hap,
            tc.tile_pool(name="yp", bufs=2) as yp,
        ):
            # PE warmup: dummy matmuls keep the PE clock ungated while the
            # x DMA lands and ramp it toward full frequency.
            dummy = wts.tile([P, 512], bf16)
            nc.vector.memset(dummy, 0.0)
            with tc.tile_pool(name="psW", bufs=1, space="PSUM") as psw:
                wps = psw.tile([P, 512], f32, tag="w")
                for _ in range(26):
                    nc.tensor.matmul(wps, lhsT=dummy[:, 0:P], rhs=dummy,
                                     start=True, stop=True)

            # ---- DMA: x bf16 (critical: stats), x fp8, weights, consts ----
            xs_b = big.tile([P, 2, N], bf16)
            xs_8 = big.tile([P, 2, N], f8)
            nc.gpsimd.dma_start(out=xs_8, in_=x8v)

            gm = wts.tile([P, 16], f32)
            nc.scalar.dma_start(out=gm, in_=gm_d)
            gt = wts.tile([16, P], f32)
            nc.scalar.dma_start(out=gt, in_=gt_d)

            st6 = stats.tile([P, 2, 8, 6], f32)
            for j in range(2):
                qeng = nc.sync if j == 0 else nc.scalar
                for qd in range(4):
                    sl = slice(qd * 1024, (qd + 1) * 1024)
                    qeng.dma_start(out=xs_b[:, j, sl], in_=xbv[:, j, sl])
                    for h in range(2):
                        sg = qd * 2 + h
                        ssl = slice(sg * 512, (sg + 1) * 512)
                        nc.vector.bn_stats(out=st6[:, j, sg, :],
                                           in_=xs_b[:, j, ssl])

            wqk_b = wts.tile([P, 2, 2 * C], bf16)
            nc.scalar.dma_start(out=wqk_b, in_=wqkv)
            wv_b = wts.tile([P, 2, C], bf16)
            nc.scalar.dma_start(out=wv_b, in_=wvv)
            wp_b = wts.tile([P, 2, C], bf16)
            nc.scalar.dma_start(out=wp_b, in_=wpv)
            bq4 = wts.tile([P, 2, 1], f32)
            nc.scalar.dma_start(out=bq4, in_=bq4v)
            bp0 = wts.tile([P, 2, 1], f32)
            nc.scalar.dma_start(out=bp0, in_=bp0v)
            ones8 = wts.tile([P, 2, P], f8)
            nc.gpsimd.memset(ones8, 1.0)
            eps_t = wts.tile([16, 1], f32)
            nc.vector.memset(eps_t, EPS)

            # ---- group stats -> AB[:, j, (mean, rstd)] ----
            AB = stats.tile([P, 2, 2], f32)
            with tc.tile_pool(name="psStat", bufs=1, space="PSUM") as psst:
                for j in range(2):
                    mv = stats.tile([P, 2], f32, tag="mv")
                    nc.vector.bn_aggr(out=mv, in_=st6[:, j])
                    t2 = stats.tile([P, 2], f32, tag="t2")
                    nc.vector.tensor_copy(out=t2[:, 0:1], in_=mv[:, 0:1])
                    nc.vector.scalar_tensor_tensor(
                        out=t2[:, 1:2], in0=mv[:, 0:1], scalar=mv[:, 0:1],
                        in1=mv[:, 1:2], op0=ALU.mult, op1=ALU.add,
                    )
                    gagg = psst.tile([16, 2], f32, tag="gagg")
                    nc.tensor.matmul(gagg, lhsT=gm, rhs=t2, start=True,
                                     stop=True)
                    grs = stats.tile([16, 2], f32, tag="grs")
                    nc.scalar.copy(out=grs[:, 0:1], in_=gagg[:, 0:1])
                    sq = stats.tile([16, 1], f32, tag="sq")
                    nc.scalar.square(out=sq, in_=gagg[:, 0:1])
                    var = stats.tile([16, 1], f32, tag="var")
                    nc.vector.tensor_sub(out=var, in0=gagg[:, 1:2], in1=sq)
                    nc.scalar.activation(out=var, in_=var, func=AF.Sqrt,
                                         bias=eps_t, scale=1.0)
                    nc.vector.reciprocal(out=grs[:, 1:2], in_=var)
                    gb = psst.tile([P, 2], f32, tag="gb")
                    nc.tensor.matmul(gb, lhsT=gt, rhs=grs, start=True,
                                     stop=True)
                    nc.scalar.copy(out=AB[:, j, :], in_=gb)
                # preload the exp table while nothing else needs ACT
                dexp = stats.tile([16, 1], f32, tag="dexp")
                nc.scalar.activation(out=dexp, in_=eps_t, func=AF.Exp)

            # ---- runtime weight scaling + bias fixups ----
            # wqk_s = wqk * r  (host wqk already carries gn_w and the x4
            # q/k fp8 scale);  wv_s = wv * r
            wqk_s = wts.tile([P, 2, 2 * C], f8)
            wv_s = wts.tile([P, 2, C], f8)
            mu8 = wts.tile([P, 2, 16], f8)
            cvs = wts.tile([P, 2, 16], bf16)
            bqe = wts.tile([P, 2, 1], f32)
            bpe = wts.tile([P, 2, 1], f32)
            for j in range(2):
                nc.vector.tensor_scalar(
                    out=wqk_s[:, j, :], in0=wqk_b[:, j, :],
                    scalar1=AB[:, j, 1:2], scalar2=None, op0=ALU.mult)
                nc.vector.tensor_scalar(
                    out=wv_s[:, j, :], in0=wv_b[:, j, :],
                    scalar1=AB[:, j, 1:2], scalar2=None, op0=ALU.mult)
                nc.vector.tensor_scalar(
                    out=mu8[:, j, 0:1], in0=AB[:, j, 0:1],
                    scalar1=MUS, scalar2=None, op0=ALU.mult)

            # bridge dummies: keep the PE clock up through the DVE scaling
            with tc.tile_pool(name="psW2", bufs=1, space="PSUM") as psw2:
                wps2 = psw2.tile([P, 512], f32, tag="w2")
                for _ in range(8):
                    nc.tensor.matmul(wps2, lhsT=dummy[:, 0:P], rhs=dummy,
                                     start=True, stop=True)

            # bias matvecs:
            #   bqe = bq4 - (wqk_s[:, :, q]^T mu)          (q bias fixup)
            #   cv  = -(wv_s^T mu);  bpe = bp0 + wp^T cv   (proj bias fixup)
            with tc.tile_pool(name="psMv", bufs=2, space="PSUM") as psmv:
                for jo in range(2):
                    mq = psmv.tile([P, 1], f32, tag="mv0")
                    nc.tensor.matmul(mq, lhsT=wqk_s[:, :, jo * P:(jo + 1) * P],
                                     rhs=mu8[:, :, 0:1], start=True, stop=True,
                                     perf_mode=DR)
                    nc.vector.tensor_scalar(
                        out=bqe[:, jo, :], in0=mq, scalar1=-1.0 / MUS,
                        scalar2=bq4[:, jo, :], op0=ALU.mult, op1=ALU.add)
                    cm = psmv.tile([P, 1], f32, tag="mv1")
                    nc.tensor.matmul(cm, lhsT=wv_s[:, :, jo * P:(jo + 1) * P],
                                     rhs=mu8[:, :, 0:1], start=True, stop=True,
                                     perf_mode=DR)
                    nc.vector.tensor_scalar(
                        out=cvs[:, jo, 0:1], in0=cm, scalar1=-1.0 / MUS,
                        scalar2=None, op0=ALU.mult)
                for jo in range(2):
                    pb = psmv.tile([P, 1], f32, tag="mv0")
                    for j in range(2):
                        nc.tensor.matmul(
                            pb, lhsT=wp_b[:, j, jo * P:(jo + 1) * P],
                            rhs=cvs[:, j, 0:1],
                            start=(j == 0), stop=(j == 1))
                    nc.vector.tensor_add(out=bpe[:, jo, :], in0=pb,
                                         in1=bp0[:, jo, :])

            # ---- main compute: k/q gen, then attention with v-gen embedded
            q_s = big.tile([P, 2, NH], f8)
            k_s = big.tile([P, 2, N], f8)
            v_s = big.tile([P, MC, C], f8)
            with (
                tc.tile_pool(name="psQK", bufs=2, space="PSUM") as psqk,
                tc.tile_pool(name="psAcc", bufs=1, space="PSUM") as psacc,
                tc.tile_pool(name="psPP", bufs=1, space="PSUM") as pspp,
            ):
                # k for all 4096 cols; q for own 2048 (interleaved so q0/k0
                # land first and attention tile 0 can start early)
                for tt in range(KT):
                    sl = slice(tt * 512, (tt + 1) * 512)
                    kq = psqk.tile([P, 2, 512], f32, name=f"kq{tt}", tag="qk")
                    for jo in range(2):
                        nc.tensor.matmul(
                            kq[:, jo, :],
                            lhsT=wqk_s[:, :, C + jo * P:C + (jo + 1) * P],
                            rhs=xs_8[:, :, sl], start=True, stop=True,
                            perf_mode=DR)
                    nc.vector.tensor_copy(out=k_s[:, :, sl], in_=kq)
                    if tt < NT:
                        qp = psqk.tile([P, 2, 512], f32, name=f"qp{tt}",
                                       tag="qk")
                        for jo in range(2):
                            nc.tensor.matmul(
                                qp[:, jo, :],
                                lhsT=wqk_s[:, :, jo * P:(jo + 1) * P],
                                rhs=xs_8[:, :, sl], start=True, stop=True,
                                perf_mode=DR)
                        for jo in range(2):
                            nc.vector.tensor_scalar(
                                out=q_s[:, jo, sl], in0=qp[:, jo, :],
                                scalar1=bqe[:, jo, :], scalar2=None,
                                op0=ALU.add)

                # ---- attention ----
                def stage1(pend):
                    av0, av1, sp, psl, st = pend
                    rb = rp.tile([P, 512], f32, name="rb", tag="rb")
                    nc.vector.reciprocal(out=rb, in_=sp)
                    ha = hap.tile([P, 2, 512], bf16, name="ha", tag="ha")
                    nc.vector.tensor_mul(out=ha[:, 0, :], in0=av0, in1=rb)
                    nc.vector.tensor_mul(out=ha[:, 1, :], in0=av1, in1=rb)
                    st["ha"] = ha

                def stage2(pend):
                    psl = pend[3]
                    ha = pend[4]["ha"]
                    yt = yp.tile([P, 2, 512], bf16, name="yt", tag="yt")
                    for jo in range(2):
                        pp = pspp.tile([P, 512], f32, name="pp", tag="pp")
                        for j in range(2):
                            nc.tensor.matmul(
                                pp, lhsT=wp_b[:, j, jo * P:(jo + 1) * P],
                                rhs=ha[:, j, :],
                                start=(j == 0), stop=(j == 1))
                        nc.vector.scalar_tensor_tensor(
                            out=yt[:, jo, :], in0=pp, scalar=bpe[:, jo, :],
                            in1=xs_b[:, jo, psl], op0=ALU.add, op1=ALU.add)
                    nc.sync.dma_start(out=yv[:, :, psl], in_=yt)

                pend = None
                for tt in range(NT):
                    sl = slice(tt * 512, (tt + 1) * 512)
                    av0 = psacc.tile([P, 512], f32, name="av0", tag="av0")
                    av1 = psacc.tile([P, 512], f32, name="av1", tag="av1")
                    sp = psacc.tile([P, 512], f32, name="sp", tag="sp")
                    ets = [None] * MCP

                    def avs(pr, av0=av0, av1=av1, sp=sp, ets=ets):
                        et = ets[pr]
                        vsl = v_s[:, 2 * pr:2 * pr + 2, :]
                        first, last = pr == 0, pr == MCP - 1
                        nc.tensor.matmul(av0, lhsT=vsl[:, :, 0:P], rhs=et,
                                         start=first, stop=last, perf_mode=DR)
                        nc.tensor.matmul(av1, lhsT=vsl[:, :, P:C], rhs=et,
                                         start=first, stop=last, perf_mode=DR)
                        nc.tensor.matmul(sp, lhsT=ones8, rhs=et,
                                         start=first, stop=last, perf_mode=DR)

                    for pr in range(MCP):
                        if pend is not None:
                            if pr == 0:
                                stage1(pend)
                            elif pr == 4:
                                stage2(pend)
                                pend = None
                        qkp = psqk.tile([P, 2, 512], f32, name="qkp",
                                        tag="qk")
                        for i in range(2):
                            mc = 2 * pr + i
                            nc.tensor.matmul(
                                qkp[:, i, :],
                                lhsT=k_s[:, :, mc * P:(mc + 1) * P],
                                rhs=q_s[:, :, sl], start=True, stop=True,
                                perf_mode=DR)
                        et = epool.tile([P, 2, 512], f8, name=f"et{pr % 3}",
                                        tag="et")
                        nc.scalar.activation(out=et, in_=qkp, func=AF.Exp,
                                             scale=1.0 / (16.0 * SQ * SQ))
                        ets[pr] = et
                        if tt == 0:
                            # v-gen embedded in tile 0 (uses the pp bank,
                            # which tile 0 never needs for stage2)
                            vt = pspp.tile([P, 512], f32, name=f"vt{pr}",
                                           tag="pp")
                            vtv = vt.rearrange("p (i c) -> p i c", c=C)
                            for i in range(2):
                                mc = 2 * pr + i
                                nc.tensor.matmul(
                                    vtv[:, i, :],
                                    lhsT=xs_8[:, :, mc * P:(mc + 1) * P],
                                    rhs=wv_s, start=True, stop=True,
                                    perf_mode=DR)
                            nc.vector.tensor_copy(
                                out=v_s[:, 2 * pr:2 * pr + 2, :], in_=vtv)
                        if pr > 0:
                            avs(pr - 1)
                    avs(MCP - 1)
                    pend = (av0, av1, sp, sl, {})
                stage1(pend)
                stage2(pend)

    nc.compile()
    return nc


def _get_prog():
    global _prog
    if _prog is None:
        _prog = _build_program()
    return _prog


def _host_prep(x, gn_w, gn_b, qkv_w, qkv_b, proj_w, proj_b):
    """Returns (shared input dict, per-core xb list, per-core x8 list)."""
    import ml_dtypes

    x = np.asarray(x, dtype=np.float32)
    gn_w = np.asarray(gn_w, dtype=np.float32)
    gn_b = np.asarray(gn_b, dtype=np.float32)
    qkv_w = np.asarray(qkv_w, dtype=np.float32)
    qkv_b = np.asarray(qkv_b, dtype=np.float32)
    proj_w = np.asarray(proj_w, dtype=np.float32)
    proj_b = np.asarray(proj_b, dtype=np.float32)

    bf = ml_dtypes.bfloat16
    f8 = ml_dtypes.float8_e4m3

    Wq = qkv_w[0:C] * gn_w[None, :]
    Wk = qkv_w[C:2 * C] * gn_w[None, :]
    Wv = qkv_w[2 * C:3 * C] * gn_w[None, :]
    bq_h = qkv_w[0:C] @ gn_b + qkv_b[0:C]
    bv_h = qkv_w[2 * C:3 * C] @ gn_b + qkv_b[2 * C:3 * C]
    bp_h = proj_b + proj_w @ bv_h

    wqk = (SQ * np.concatenate([Wq.T, Wk.T], axis=1)).astype(bf)  # [C, 2C]
    wv_h = np.ascontiguousarray(Wv.T).astype(bf)
    wp_h = np.ascontiguousarray(proj_w.T).astype(bf)

    cidx = np.arange(P)
    gm = np.zeros((P, 16), dtype=np.float32)
    gm[cidx, cidx // GSIZE] = 1.0 / GSIZE
    gt = np.zeros((16, P), dtype=np.float32)
    gt[cidx // GSIZE, cidx] = 1.0

    shared = {
        "wqk": wqk,
        "wv": wv_h,
        "wp": wp_h,
        "bq4": (SQ * bq_h).reshape(C, 1).astype(np.float32),
        "bp0": bp_h.reshape(C, 1).astype(np.float32),
        "gm": gm,
        "gt": gt,
    }

    xf = x.reshape(B, C, N)
    xb_per_core = []
    x8_per_core = []
    for core in range(NCORES):
        b, half = core // 2, core % 2
        if half == 0:
            xc = xf[b]
        else:
            xc = np.concatenate([xf[b][:, NH:], xf[b][:, :NH]], axis=1)
        xb_per_core.append(np.ascontiguousarray(xc).astype(bf))
        x8_per_core.append(
            np.clip(np.ascontiguousarray(xc), -240, 240).astype(f8))
    return shared, xb_per_core, x8_per_core


def run_sharded(inputs, trace=False, trace_kwargs=None):
    """Run the 8-core kernel. Returns (full_output, BassKernelResults)."""
    from concourse.bass_utils import run_bass_kernel_spmd

    nc = _get_prog()
    shared, xb_per_core, x8_per_core = _host_prep(**inputs)
    in_maps = [{**shared, "xb": xb_per_core[c], "x8": x8_per_core[c]}
               for c in range(NCORES)]
    kw = {}
    if trace:
        kw["trace"] = True
        if trace_kwargs:
            kw["trace_kwargs"] = trace_kwargs
    res = run_bass_kernel_spmd(nc, in_maps, list(range(NCORES)), **kw)

    out = np.empty((B, C, N), dtype=np.float32)
    for core in range(NCORES):
        b, half = core // 2, core % 2
        yc = np.asarray(res.results[core]["y"], dtype=np.float32)
        out[b][:, half * NH:(half + 1) * NH] = yc
    return out.reshape(B, C, HH, WW), res


def kernel(**inputs):
    out, _ = run_sharded(inputs)
    return out
